# revision 24
# baseline (speedup 1.0000x reference)
"""KitNET anomaly-detection ensemble (25 tiny tied-weight autoencoders) on 8 Trainium2 cores.

Strategy (v4, packed-AE, 2-wave pipelined):
  - Data-parallel over batch: each of the 8 cores processes B/8 = 16384 samples.
  - The per-AE feature gather x[:, idx] is applied on the host as a column
    permutation of x (pure data marshaling, one gather pass). Packed column
    16*a + k holds natural feature idx[a, k], so each AE's 16 features are
    contiguous and AE pair p = (2p, 2p+1) occupies a 32-aligned stripe.
  - Encode/decode are BLOCK-DIAGONAL: 13 small matmuls on distinct 32x32 PE
    sub-array cells (tile_position) running concurrently, split into two
    waves of <=8 so each wave's outputs fill a [128, 2banks, 512] psum tile
    with bank-aligned, strip-distinct slots (the has_written-safe pattern).
  - The 2-bank wave tiles come from a bufs=2 pool, so consecutive tiles
    pipeline: PSUM = 2 (xt transpose) + 4 (wave tiles) + 2 (S) = 8 banks.
  - Sigmoids ride ACT as ONE merged [128, 2, 512] instruction per wave when
    biases are all-zero (checked on host; per-bank bias-AP instrs otherwise).
  - err^2 split DVE / GPSIMD; sqrt + 25-AE sum phase-split to the end.
"""

import sys

for _p in ("/opt/trn_rl_repo", "/opt/pypackages"):
    if _p not in sys.path:
        sys.path.append(_p)

import numpy as np

B = 131072
F = 400          # features
N_AE = 25
KF = 16          # features per AE
H = 12           # hidden per AE
EPS = 1e-6
N_CORES = 8
BC = B // N_CORES    # 16384 samples per core
NB = 512             # batch tile (matmul moving free dim)
NT = BC // NB        # 32 tiles per core
NPAIR = 13           # 12 pairs of AEs + AE 24 alone

_NC_CACHE = {}


def _xt_geom(p):
    """xt layout: pair p at partitions [32*(p%4), +32) of chunk p//4
    (pair 12 = AE24: partitions [0,16) of chunk 3). Returns (rows, r, chunk)."""
    if p < 12:
        return 32, 32 * (p % 4), p // 4
    return 16, 0, 3


def _enc_geom(p):
    """Encode wave/slot for pair p: returns (wave, slice, col_strip).
    Hidden of pair p lands at psum partitions [32*col_strip, +32) of
    bank-slice `slice` of wave tile `wave`; PE cell = (xt row, col_strip)."""
    if p < 4:
        return 0, 0, p
    if p < 8:
        return 0, 1, (p - 3) % 4
    if p < 12:
        return 1, 0, p - 8
    return 1, 1, 1


def _build_nc(biases_zero):
    import concourse.tile as tile
    from concourse import bacc, mybir

    f32 = mybir.dt.float32
    bf16 = mybir.dt.bfloat16
    AF = mybir.ActivationFunctionType

    nc = bacc.Bacc()

    x_d = nc.declare_dram_parameter("x", [BC, F], f32, isOutput=False)
    wenc_d = nc.declare_dram_parameter("wenc", [128, 4, 32], bf16, isOutput=False)
    wdec_d = nc.declare_dram_parameter("wdec", [128, 4, 32], bf16, isOutput=False)
    g_d = nc.declare_dram_parameter("gmat", [128, 4, 32], bf16, isOutput=False)
    hb_d = nc.declare_dram_parameter("hbm", [128, 4], f32, isOutput=False)
    vb_d = nc.declare_dram_parameter("vbm", [128, 4], f32, isOutput=False)
    id_d = nc.declare_dram_parameter("ident", [128, 128], bf16, isOutput=False)
    y_d = nc.declare_dram_parameter("y", [BC], f32, isOutput=True)

    with tile.TileContext(nc) as tc:
        with (
            tc.tile_pool(name="singles", bufs=1) as singles,
            tc.tile_pool(name="xnat", bufs=3) as xnat_p,
            tc.tile_pool(name="xb", bufs=3) as xb_p,
            tc.tile_pool(name="xt", bufs=3) as xt_p,
            tc.tile_pool(name="ht", bufs=2) as ht_p,
            tc.tile_pool(name="rec", bufs=3) as rec_p,
            tc.tile_pool(name="err", bufs=2) as err_p,
            tc.tile_pool(name="xtp", bufs=2, space="PSUM") as xtp_p,
            tc.tile_pool(name="edp", bufs=2, space="PSUM") as edp_p,
            tc.tile_pool(name="sp", bufs=2, space="PSUM") as sp_p,
        ):
            # --- constants ---
            ident = singles.tile([128, 128], bf16)
            nc.sync.dma_start(out=ident, in_=id_d[:, :])
            wenc_sb = singles.tile([128, 4, 32], bf16)
            nc.sync.dma_start(out=wenc_sb, in_=wenc_d[:, :, :])
            wdec_sb = singles.tile([128, 4, 32], bf16)
            nc.sync.dma_start(out=wdec_sb, in_=wdec_d[:, :, :])
            g_sb = singles.tile([128, 4, 32], bf16)
            nc.sync.dma_start(out=g_sb, in_=g_d[:, :, :])
            hb_sb = singles.tile([128, 4], f32)
            nc.sync.dma_start(out=hb_sb, in_=hb_d[:, :])
            vb_sb = singles.tile([128, 4], f32)
            nc.sync.dma_start(out=vb_sb, in_=vb_d[:, :])
            # per-AE squared-error sums, 4 tiles stacked on partition strips:
            # sall[32*(t%4) + a, t//4, i]
            sall = singles.tile([128, NT // 4, NB], f32)

            x_ap = x_d[:, :]

            # Software pipeline: iteration t emits tile t's head (load,
            # transpose, encode, decode) and tile t-1's tail (err^2, G,
            # S copy-out), so every tail dependency is a full tile old and
            # no engine stalls on a same-tile producer.
            live = {}

            def head(t):
                g = t % 4
                # ---- load 512 samples: [128p, 4sb, 400f]
                xn = xnat_p.tile([128, 4, F], f32, tag="xn")
                nc.sync.dma_start(
                    out=xn,
                    in_=x_ap[t * NB:(t + 1) * NB, :].rearrange(
                        "(s p) f -> p s f", p=128
                    ),
                )
                xnb = xb_p.tile([128, 4, F], bf16, tag="xnb")
                nc.vector.tensor_copy(out=xnb, in_=xn)

                # ---- transpose to packed feature-major xt[fp, chunk, i]
                xt = xt_p.tile([128, 4, NB], bf16, tag="xt")
                for half in range(2):
                    pxt = xtp_p.tile([128, 2, NB], bf16, tag="xtp")
                    for cc in range(2):
                        c = 2 * half + cc
                        fw = 128 if c < 3 else 16
                        for sb in range(4):
                            nc.tensor.transpose(
                                pxt[0:fw, cc, sb * 128:(sb + 1) * 128],
                                xnb[:, sb, c * 128:c * 128 + fw],
                                ident,
                            )
                    if half == 0:
                        nc.scalar.copy(out=xt[:, 0:2, :], in_=pxt)
                    else:
                        nc.vector.tensor_copy(out=xt[:, 2:4, :], in_=pxt)

                # ---- encode + hidden sigmoid, two waves
                ht = ht_p.tile([128, 4, NB], bf16, tag="ht")
                for w in range(2):
                    ew = edp_p.tile([128, 2, NB], f32, tag="ed")
                    for p in (range(8) if w == 0 else range(8, NPAIR)):
                        rows, xt_r, xt_c = _xt_geom(p)
                        _, s_e, c_e = _enc_geom(p)
                        nc.tensor.matmul(
                            ew[32 * c_e:32 * c_e + 32, s_e, :],
                            lhsT=wenc_sb[xt_r:xt_r + rows, xt_c, :],
                            rhs=xt[xt_r:xt_r + rows, xt_c, :],
                            start=True,
                            stop=True,
                            tile_position=(xt_r, 32 * c_e),
                        )
                    # hidden of wave w -> ht[:, 2w + slice, :]
                    if biases_zero:
                        nc.scalar.activation(
                            out=ht[:, 2 * w:2 * w + 2, :], in_=ew, func=AF.Sigmoid
                        )
                    else:
                        for s in range(2):
                            nc.scalar.activation(
                                out=ht[:, 2 * w + s, :],
                                in_=ew[:, s, :],
                                func=AF.Sigmoid,
                                bias=hb_sb[:, 2 * w + s:2 * w + s + 1],
                            )

                # ---- decode + rec sigmoid, two waves (by rec chunk pair)
                rec = rec_p.tile([128, 4, NB], bf16, tag="rec")
                for w in range(2):
                    dw = edp_p.tile([128, 2, NB], f32, tag="ed")
                    for p in (range(8) if w == 0 else range(8, NPAIR)):
                        rows, xt_r, xt_c = _xt_geom(p)
                        we, s_e, c_e = _enc_geom(p)
                        ow = 32 if p < 12 else 16
                        # rec slot: chunk = p//4 -> (wave p//8, slice (p//4)%2)
                        d_s = (p // 4) % 2
                        d_c = p % 4 if p < 12 else 0
                        nc.tensor.matmul(
                            dw[32 * d_c:32 * d_c + ow, d_s, :],
                            lhsT=wdec_sb[32 * c_e:32 * c_e + rows,
                                         2 * we + s_e, 0:ow],
                            rhs=ht[32 * c_e:32 * c_e + rows, 2 * we + s_e, :],
                            start=True,
                            stop=True,
                            tile_position=(32 * c_e, 32 * d_c),
                        )
                    if biases_zero:
                        nc.scalar.activation(
                            out=rec[:, 2 * w:2 * w + 2, :], in_=dw, func=AF.Sigmoid
                        )
                    else:
                        for s in range(2):
                            nc.scalar.activation(
                                out=rec[:, 2 * w + s, :],
                                in_=dw[:, s, :],
                                func=AF.Sigmoid,
                                bias=vb_sb[:, 2 * w + s:2 * w + s + 1],
                            )

                live[t] = (xt, rec)

            def tail(t):
                g = t % 4
                xt, rec = live.pop(t)
                # ---- err^2, out-of-place to keep DVE 2x perf mode
                err = err_p.tile([128, 4, NB], bf16, tag="err")
                nc.vector.tensor_sub(err, xt, rec)
                nc.vector.tensor_mul(rec, err, err)

                # ---- per-AE sums: S[32*(t%4) + a, i] += G^T @ err2
                if g == 0:
                    ps4 = sp_p.tile([128, NB], f32, tag="sp", name="ps4")
                    live["sp"] = ps4
                else:
                    ps4 = live["sp"]
                for c in range(4):
                    kw = 128 if c < 3 else 16
                    nc.tensor.matmul(
                        ps4[32 * g:32 * (g + 1), :],
                        lhsT=g_sb[0:kw, c, :],
                        rhs=rec[0:kw, c, :],
                        start=(c == 0),
                        stop=(c == 3),
                        tile_position=(0, 32 * g),
                    )
                if g == 3:
                    nc.vector.tensor_copy(out=sall[:, t // 4, :], in_=ps4)

            for t in range(NT + 1):
                if t < NT:
                    head(t)
                if t >= 1:
                    tail(t - 1)

            # ---- phase B: rmse = sqrt(S/16 + eps); y = sum over AEs
            eps_sb = singles.tile([128, 1], f32)
            nc.vector.memset(eps_sb, EPS)
            nc.scalar.activation(
                out=sall, in_=sall, func=AF.Sqrt, bias=eps_sb, scale=1.0 / KF
            )
            ones4 = singles.tile([128, 4], f32)
            nc.gpsimd.memset(ones4, 0.0)
            for g in range(4):
                nc.gpsimd.memset(ones4[32 * g:32 * g + N_AE, g:g + 1], 1.0)
            ybuf = singles.tile([4, NT // 4, NB], f32)
            for j in range(NT // 4):
                py = sp_p.tile([4, NB], f32, tag="sp")
                nc.tensor.matmul(
                    py,
                    lhsT=ones4,
                    rhs=sall[:, j, :],
                    start=True,
                    stop=True,
                )
                if j % 2 == 0:
                    nc.vector.tensor_copy(out=ybuf[:, j, :], in_=py)
                else:
                    nc.scalar.copy(out=ybuf[:, j, :], in_=py)
            # y[b], b = t*NB + i, t = 4j + g  ->  y view [g, j, i]
            y_ap = y_d[:].rearrange("(j g i) -> g j i", g=4, i=NB)
            nc.sync.dma_start(out=y_ap, in_=ybuf)

    nc.compile()
    return nc


def _host_mats(W, hb, vb, idx):
    import ml_dtypes

    bf16 = ml_dtypes.bfloat16
    W = np.asarray(W, np.float32)
    hb = np.asarray(hb, np.float32)
    vb = np.asarray(vb, np.float32)
    idx = np.asarray(idx)

    wenc = np.zeros((128, 4, 32), np.float32)
    wdec = np.zeros((128, 4, 32), np.float32)
    gmat = np.zeros((128, 4, 32), np.float32)
    hbm = np.zeros((128, 4), np.float32)
    vbm = np.zeros((128, 4), np.float32)

    for a in range(N_AE):
        p, half = a // 2, a % 2
        rows, xt_r, xt_c = _xt_geom(p)
        w, s_e, c_e = _enc_geom(p)
        d_bank = p // 4  # rec chunk = vbm column
        for k in range(KF):
            for h in range(H):
                wenc[xt_r + 16 * half + k, xt_c, 12 * half + h] = W[a, k, h]
                wdec[32 * c_e + 12 * half + h, 2 * w + s_e, 16 * half + k] = W[a, k, h]
            gmat[xt_r + 16 * half + k, xt_c, a] = 1.0
            vbm[xt_r + 16 * half + k, d_bank] = vb[a, k]
        for h in range(H):
            hbm[32 * c_e + 12 * half + h, 2 * w + s_e] = hb[a, h]

    return {
        "wenc": wenc.astype(bf16),
        "wdec": wdec.astype(bf16),
        "gmat": gmat.astype(bf16),
        "hbm": hbm,
        "vbm": vbm,
        "ident": np.eye(128, dtype=np.float32).astype(bf16),
    }, bool(not (np.any(hb) or np.any(vb)))


def _get_nc(biases_zero):
    key = ("nc", biases_zero)
    if key not in _NC_CACHE:
        _NC_CACHE[key] = _build_nc(biases_zero)
    return _NC_CACHE[key]


def _run(x, W, hb, vb, idx, trace=False):
    from concourse.bass_utils import run_bass_kernel_spmd

    idx = np.asarray(idx)
    # host-side gather: packed column 16a+k = natural feature idx[a, k]
    xg = np.ascontiguousarray(np.asarray(x, np.float32)[:, idx.reshape(-1)])
    consts, biases_zero = _host_mats(W, hb, vb, idx)
    in_maps = [
        {"x": xg[c * BC:(c + 1) * BC], **consts} for c in range(N_CORES)
    ]
    nc = _get_nc(biases_zero)
    res = run_bass_kernel_spmd(nc, in_maps, list(range(N_CORES)), trace=trace)
    y = np.concatenate([res.results[c]["y"] for c in range(N_CORES)])
    return y, res


def kernel(x, W, hb, vb, idx):
    y, _ = _run(x, W, hb, vb, idx)
    return y
